# revision 37
# baseline (speedup 1.0000x reference)
"""Conv2d(32->32, 3x3, stride 1, pad 1) on X[32,32,224,224] fp32, data-parallel
over 8 NeuronCores (4 images per core).

Per-core algorithm ("full-K row-rotated")
-----------------------------------------
The conv is computed as full-array PE matmuls with contraction K = 128 =
(q in 0..3 row-taps) x (c = 32 input channels), M = 64 = (ho in 0..1) x
(k = 32 output channels), N = 448 = (u in 0..1 row-pairs) x (w in 0..223),
in fp16 (1 column/cycle, warms the PE clock gate; fp32 runs 4x slower and
float32r runs cold at 1.2 GHz and overlaps poorly).

X (host-padded to 226 wide, host-cast to fp16, host-rotated so row 4*jd + q
sits at partition group q) is DMA'd per H-slice as Xr0; a second copy Xr1,
rotated down by two rows (j = 4*jd + q + 2), is built on-chip by two
SBUF->SBUF partition-remap DMAs per xr0 half (engine time but no HBM
bandwidth, which the startup is short on).  For an output pair starting at
even hb0, the 4 input rows sit at partition group q, one free offset -- so
one matmul contracts all 4 row-taps at once; even pairs read Xr0, odd
pairs Xr1.  One PSUM accumulation group = 3 matmuls (s = column
shift of the rhs into the padded row).  Per core: 672 matmuls of
[128,64]x[128,448].

lhsT[32*q+c, s][32*ho+k] = W[k, c, q-ho, s] (zero outside 0<=r<3), so both
output rows of an hb are produced per matmul.  Bias is fused into the
PSUM->SBUF eviction (ScalarE/VectorE alternating), which also casts to fp16.
Work is H-sliced (112 output rows) for SBUF fit and load/compute overlap.

Y leaves the device in the staged layout [n, G, k, m, w] (fp16) where the
output row h = 4*m + G -- each store is then one >=6KB descriptor per
partition instead of 896B per (k, m) row, which otherwise saturates all 16
DMA queues with descriptor overhead.  The host un-interleaves + casts fp32.
"""

import sys

import numpy as np

try:
    import concourse.bass as bass  # noqa: F401
except ImportError:  # pragma: no cover
    sys.path.insert(0, "/opt/trn_rl_repo")

import ml_dtypes
import concourse.mybir as mybir
import concourse.tile as tile
from concourse import bacc
from concourse.bass_utils import run_bass_kernel_spmd

NCORES = 8
NB = 4  # images per core
C = 32
K = 32
H = 224
W = 224
WP = 226  # padded width
NQ = 57  # row-quads in the host-rotated layout (228 padded rows / 4)
RS = 112  # output rows per slice
NSLICE = H // RS
NJD = RS // 4 + 1  # row-quads per rotated slice tile
F32 = mybir.dt.float32
F16 = mybir.dt.float16
AF = mybir.ActivationFunctionType
_NP16 = np.float16


def set_dtype(name):
    """'fp16' (default) or 'bf16' for the matmul operand precision."""
    global F16, _NP16, _NC
    if name == "bf16":
        F16, _NP16 = mybir.dt.bfloat16, ml_dtypes.bfloat16
    else:
        F16, _NP16 = mybir.dt.float16, np.float16
    _NC = None


def conv_body(tc, X, Wt, Bias, Y):
    nc = tc.nc
    with (
        tc.tile_pool(name="const", bufs=1) as cpool,
        tc.tile_pool(name="xpool", bufs=3) as xpool,
        tc.tile_pool(name="ypool", bufs=4) as ypool,
        tc.tile_pool(name="tpool", bufs=4) as tpool,
        tc.tile_pool(name="ppool", bufs=8, space="PSUM") as ppool,
    ):
        wt_sb = cpool.tile([128, 384], F16)
        nc.sync.dma_start(out=wt_sb[:], in_=Wt)
        # warm the PE clock-gate while the first X tile loads; the warm tile
        # is engine-written (memset), so the matmuls start ~3us before the
        # weight DMA's HBM completion receipt would allow
        warm = cpool.tile([128, 448], F16)
        nc.vector.memset(warm[:], 1.0)
        pw = ppool.tile([64, 448], F32, name="pw", tag="pt")
        for _ in range(16):
            nc.tensor.matmul(
                pw[:, :],
                warm[:, 0:64],
                warm[:, :],
                start=True,
                stop=True,
            )

        NP = RS // 8  # pairs per parity per slice
        MH = RS // 8  # m-columns per store half
        for n in range(NB):
            for t in range(NSLICE):
                # (jd w) merge into one >=6KB descriptor per partition.
                xr0 = xpool.tile([128, NJD, WP], F16, name="xr0", tag="xr0")
                jq0 = (NJD - 1) * t
                # split loads: with packet-round-robin across in-flight DMAs,
                # smaller pieces make the earliest-needed quads land sooner
                xr1 = xpool.tile([128, NJD - 1, WP], F16, name="xr1", tag="xr1")
                halves = ((0, 14), (14, NJD))
                for a, b in halves:
                    nc.sync.dma_start(
                        out=xr0[:, a:b, :],
                        in_=X[n, :, :, jq0 + a : jq0 + b, :],
                    )
                # xr1 (the same rows rotated down by two) is built on-chip,
                # piece-wise behind each xr0 half: SBUF->SBUF costs engine
                # time but no HBM bandwidth, which the startup (this tile +
                # next tile's prefetch) is short on.  Issued after both
                # loads so the ring-FIFO semaphore wait of a copy never
                # delays a load's descriptor generation.
                for a, b in halves:
                    nc.sync.dma_start(
                        out=xr1[0:64, a : min(b, NJD - 1), :],
                        in_=xr0[64:128, a : min(b, NJD - 1), :],
                    )
                    lo = max(a - 1, 0)
                    nc.sync.dma_start(
                        out=xr1[64:128, lo : b - 1, :],
                        in_=xr0[0:64, lo + 1 : b, :],
                    )

                # staging for the whole slice: partition group G = 2*pi + ho,
                # free (m_local = 2i + u, w); fp16 halves the store bytes
                ysb = ypool.tile([128, RS // 4, 224], F16, name="ysb", tag="ysb")
                for pi in range(2):
                    for i in range(NP):  # pair (hbl0, hbl0+2), hbl0 = 4i + pi
                        src = xr0 if pi == 0 else xr1
                        jd0 = 2 * i  # local free index of u=0 in xr0/xr1
                        dst = ysb[64 * pi : 64 * (pi + 1), 2 * i : 2 * i + 2, :]
                        # Hybrid: even pairs pack s in {0,1} into M=128 (full
                        # PE width, one stream of the whole 226-col window,
                        # s=2 accumulated into the sg=0 half at a -2 shift) --
                        # their eviction is a DVE shifted add of the two
                        # halves.  Odd pairs use the M=64 3-matmul form whose
                        # eviction is a 1-input ACT copy.  Only DVE/ACT can
                        # read PSUM, and only DVE takes two tensors, so this
                        # splits eviction work across both while cutting PE
                        # columns by 1/3 on even pairs.  Bias is host-added.
                        if i % 2 == 0:
                            pt = ppool.tile([128, 2, 226], F32, name="pt", tag="pt")
                            nc.tensor.matmul(
                                pt[:, :, :],
                                wt_sb[:, 0:128],
                                src[:, jd0 : jd0 + 2, 0:226],
                                start=True,
                                stop=False,
                            )
                            nc.tensor.matmul(
                                pt[0:64, :, 0:224],
                                wt_sb[:, 128:192],
                                src[:, jd0 : jd0 + 2, 2:226],
                                start=False,
                                stop=True,
                                skip_group_check=True,
                            )
                            # PSUM has a single DVE read port, so the two
                            # halves can't be added in one op: ACT stages the
                            # shifted sg1 half to SBUF, DVE adds sg0
                            tmp = tpool.tile(
                                [64, 2, 224], F32, name="tmp", tag="tmp"
                            )
                            nc.scalar.activation(
                                tmp[:, :, :], pt[64:128, :, 1:225], AF.Copy
                            )
                            nc.vector.tensor_add(
                                dst,
                                pt[0:64, :, 0:224],
                                tmp[:, :, :],
                            )
                        else:
                            pt = ppool.tile([64, 2, 224], F32, name="pt1", tag="pt")
                            for s in range(3):
                                nc.tensor.matmul(
                                    pt[:, :, :],
                                    wt_sb[:, 192 + 64 * s : 256 + 64 * s],
                                    src[:, jd0 : jd0 + 2, s : s + 224],
                                    start=(s == 0),
                                    stop=(s == 2),
                                )
                            if (i // 2) % 2 == 0:
                                nc.scalar.activation(dst, pt[:, :, :], AF.Copy)
                            else:
                                nc.vector.tensor_copy(dst, pt[:, :, :])
                        # store finished m-halves while later pairs compute;
                        # (m, w) merge into one >=6KB descriptor per partition
                        # on the (otherwise idle) ACT ring: gpsimd now does
                        # half the evictions and sync has the loads.  The
                        # very last parity goes out in finer chunks on the
                        # (idle by then) sync ring to shorten the drain tail.
                        mtail = (
                            (MH, 20, 2 * MH)
                            if (pi and n == NB - 1 and t == NSLICE - 1)
                            else (MH, 2 * MH)
                        )
                        if 2 * i + 2 in mtail:
                            ci = mtail.index(2 * i + 2)
                            mlo = (0,) + mtail[:-1]
                            mlo, mhi = mlo[ci], mtail[ci]
                            eng = nc.sync if len(mtail) == 3 and ci else nc.scalar
                            eng.dma_start(
                                out=Y[n, 2 * pi : 2 * pi + 2, :,
                                      RS // 4 * t + mlo : RS // 4 * t + mhi, :],
                                in_=ysb[64 * pi : 64 * (pi + 1), mlo : mhi, :],
                            )


def build_nc(nb=NB, repeat=1):
    assert nb == NB
    nc = bacc.Bacc("TRN2", target_bir_lowering=False, debug=False)
    # X[n, q, c, jq, w] holds padded row 4*jq + q
    X = nc.dram_tensor("X", [NB, 4, C, NQ, WP], F16, kind="ExternalInput").ap()
    Wt = nc.dram_tensor("Wt", [128, 384], F16, kind="ExternalInput").ap()
    Bias = nc.dram_tensor("bias", [128, 1], F32, kind="ExternalInput").ap()
    # staged fp16 output: Y[n, G, k, m, w] = conv(n, k, 4*m + G, w)
    Y = nc.dram_tensor("Y", [NB, 4, K, H // 4, W], F16, kind="ExternalOutput").ap()
    with tile.TileContext(nc) as tc:
        if repeat == 1:
            conv_body(tc, X, Wt, Bias, Y)
        else:
            with tc.For_i(0, repeat, 1):
                conv_body(tc, X, Wt, Bias, Y)
    nc.compile()
    return nc


def prep_weights(Wf, b):
    """cols 0:128   Wt[:, 64*sg + 32*ho + k] = W[k, c, q-ho, sg], sg in {0,1}
    cols 128:192    Wt[:, 128 + 32*ho + k] = W[k, c, q-ho, 2]
    cols 192:384    Wt[:, 192 + 64*s + 32*ho + k] = W[k, c, q-ho, s]
    (zero outside 0<=r<3); partition = 32*q + c."""
    Wf = np.asarray(Wf, np.float32)
    Wt = np.zeros((128, 384), np.float32)
    for q in range(4):
        for ho in range(2):
            r = q - ho
            if 0 <= r <= 2:
                for sg in range(2):
                    Wt[32 * q : 32 * q + 32,
                       64 * sg + 32 * ho : 64 * sg + 32 * ho + 32] = Wf[
                        :, :, r, sg
                    ].transpose(1, 0)
                Wt[32 * q : 32 * q + 32, 128 + 32 * ho : 128 + 32 * ho + 32] = Wf[
                    :, :, r, 2
                ].transpose(1, 0)
                for s in range(3):
                    Wt[32 * q : 32 * q + 32,
                       192 + 64 * s + 32 * ho : 192 + 64 * s + 32 * ho + 32] = Wf[
                        :, :, r, s
                    ].transpose(1, 0)
    bias = np.tile(np.asarray(b, np.float32), 4).reshape(128, 1)
    return Wt.astype(_NP16), bias


def pad_input(X):
    """Pad to 228x226 and pre-rotate rows: out[n, q, c, jd, w] = row 4*jd + q."""
    X = np.ascontiguousarray(X, np.float32)
    Xp = np.zeros((X.shape[0], C, H + 4, WP), _NP16)
    Xp[:, :, 1 : H + 1, 1 : W + 1] = X
    Xr = Xp.reshape(X.shape[0], C, NQ, 4, WP).transpose(0, 3, 1, 2, 4)
    return np.ascontiguousarray(Xr)


_NC = None


def _get_nc():
    global _NC
    if _NC is None:
        _NC = build_nc(NB)
    return _NC


def kernel(X, W, b, _trace=False):
    Xp = pad_input(X)
    Wt, bias = prep_weights(W, b)
    nc = _get_nc()
    in_maps = [
        {"X": Xp[NB * c : NB * (c + 1)], "Wt": Wt, "bias": bias} for c in range(NCORES)
    ]
    res = run_bass_kernel_spmd(nc, in_maps, list(range(NCORES)), trace=_trace)
    # un-interleave the staged layout: Y[n, k, 4*m + G, w] = staged[n, G, k, m, w]
    staged = np.concatenate([res.results[c]["Y"] for c in range(NCORES)], axis=0)
    out = np.ascontiguousarray(
        staged.transpose(0, 2, 3, 1, 4).reshape(NCORES * NB, 32, 224, 224),
        dtype=np.float32,
    )
    out += np.asarray(b, np.float32)[None, :, None, None]
    if _trace:
        return out, res
    return out


# revision 38
# speedup vs baseline: 1.1794x; 1.1794x over previous
"""Conv2d(32->32, 3x3, stride 1, pad 1) on X[32,32,224,224] fp32, data-parallel
over 8 NeuronCores (4 images per core).

Per-core algorithm ("full-K row-rotated")
-----------------------------------------
The conv is computed as full-array PE matmuls with contraction K = 128 =
(q in 0..3 row-taps) x (c = 32 input channels), M = 64 = (ho in 0..1) x
(k = 32 output channels), N = 448 = (u in 0..1 row-pairs) x (w in 0..223),
in fp16 (1 column/cycle, warms the PE clock gate; fp32 runs 4x slower and
float32r runs cold at 1.2 GHz and overlaps poorly).

X (host-padded to 226 wide, host-cast to fp16, host-rotated so row 4*jd + q
sits at partition group q) is DMA'd per H-slice as Xr0; a second copy Xr1,
rotated down by two rows (j = 4*jd + q + 2), is built on-chip by two
SBUF->SBUF partition-remap DMAs per xr0 half (engine time but no HBM
bandwidth, which the startup is short on).  For an output pair starting at
even hb0, the 4 input rows sit at partition group q, one free offset -- so
one matmul contracts all 4 row-taps at once; even pairs read Xr0, odd
pairs Xr1.  One PSUM accumulation group = 3 matmuls (s = column
shift of the rhs into the padded row).  Per core: 672 matmuls of
[128,64]x[128,448].

lhsT[32*q+c, s][32*ho+k] = W[k, c, q-ho, s] (zero outside 0<=r<3), so both
output rows of an hb are produced per matmul.  Bias is fused into the
PSUM->SBUF eviction (ScalarE/VectorE alternating), which also casts to fp16.
Work is H-sliced (112 output rows) for SBUF fit and load/compute overlap.

Y leaves the device in the staged layout [n, G, k, m, w] (fp16) where the
output row h = 4*m + G -- each store is then one >=6KB descriptor per
partition instead of 896B per (k, m) row, which otherwise saturates all 16
DMA queues with descriptor overhead.  The host un-interleaves + casts fp32.
"""

import sys

import numpy as np

try:
    import concourse.bass as bass  # noqa: F401
except ImportError:  # pragma: no cover
    sys.path.insert(0, "/opt/trn_rl_repo")

import ml_dtypes
import concourse.mybir as mybir
import concourse.tile as tile
from concourse import bacc
from concourse.bass_utils import run_bass_kernel_spmd

NCORES = 8
NB = 4  # images per core
C = 32
K = 32
H = 224
W = 224
WP = 226  # padded width
NQ = 57  # row-quads in the host-rotated layout (228 padded rows / 4)
RS = 112  # output rows per slice
NSLICE = H // RS
NJD = RS // 4 + 1  # row-quads per rotated slice tile
F32 = mybir.dt.float32
F16 = mybir.dt.float16
AF = mybir.ActivationFunctionType
_NP16 = np.float16


def set_dtype(name):
    """'fp16' (default) or 'bf16' for the matmul operand precision."""
    global F16, _NP16, _NC
    if name == "bf16":
        F16, _NP16 = mybir.dt.bfloat16, ml_dtypes.bfloat16
    else:
        F16, _NP16 = mybir.dt.float16, np.float16
    _NC = None


def conv_body(tc, X, Wt, Bias, Y):
    nc = tc.nc
    with (
        tc.tile_pool(name="const", bufs=1) as cpool,
        tc.tile_pool(name="xpool", bufs=3) as xpool,
        tc.tile_pool(name="ypool", bufs=4) as ypool,
        tc.tile_pool(name="ppool", bufs=8, space="PSUM") as ppool,
    ):
        wt_sb = cpool.tile([128, 3, 64], F16)
        nc.sync.dma_start(out=wt_sb[:], in_=Wt)
        b_sb = cpool.tile([128, 1], F32)

        # warm the PE clock-gate while the first X tile loads; the warm tile
        # is engine-written (memset), so the matmuls start ~3us before the
        # weight DMA's HBM completion receipt would allow
        warm = cpool.tile([128, 448], F16)
        nc.vector.memset(warm[:], 1.0)
        pw = ppool.tile([64, 448], F32, name="pw", tag="pt")
        for _ in range(16):
            nc.tensor.matmul(
                pw[:, :],
                warm[:, 0:64],
                warm[:, :],
                start=True,
                stop=True,
            )

        NP = RS // 8  # pairs per parity per slice
        MH = RS // 8  # m-columns per store half
        for n in range(NB):
            for t in range(NSLICE):
                # (jd w) merge into one >=6KB descriptor per partition.
                xr0 = xpool.tile([128, NJD, WP], F16, name="xr0", tag="xr0")
                jq0 = (NJD - 1) * t
                # split loads: with packet-round-robin across in-flight DMAs,
                # smaller pieces make the earliest-needed quads land sooner
                xr1 = xpool.tile([128, NJD - 1, WP], F16, name="xr1", tag="xr1")
                halves = ((0, 14), (14, NJD))
                for a, b in halves:
                    nc.sync.dma_start(
                        out=xr0[:, a:b, :],
                        in_=X[n, :, :, jq0 + a : jq0 + b, :],
                    )
                if n == 0 and t == 0:
                    # bias isn't needed until the first eviction
                    nc.sync.dma_start(out=b_sb[:], in_=Bias)
                # xr1 (the same rows rotated down by two) is built on-chip,
                # piece-wise behind each xr0 half: SBUF->SBUF costs engine
                # time but no HBM bandwidth, which the startup (this tile +
                # next tile's prefetch) is short on.  Issued after both
                # loads so the ring-FIFO semaphore wait of a copy never
                # delays a load's descriptor generation.
                for a, b in halves:
                    nc.sync.dma_start(
                        out=xr1[0:64, a : min(b, NJD - 1), :],
                        in_=xr0[64:128, a : min(b, NJD - 1), :],
                    )
                    lo = max(a - 1, 0)
                    nc.sync.dma_start(
                        out=xr1[64:128, lo : b - 1, :],
                        in_=xr0[0:64, lo + 1 : b, :],
                    )

                # staging for the whole slice: partition group G = 2*pi + ho,
                # free (m_local = 2i + u, w); fp16 halves the store bytes
                ysb = ypool.tile([128, RS // 4, 224], F16, name="ysb", tag="ysb")
                for pi in range(2):
                    for i in range(NP):  # pair (hbl0, hbl0+2), hbl0 = 4i + pi
                        src = xr0 if pi == 0 else xr1
                        jd0 = 2 * i  # local free index of u=0 in xr0/xr1
                        pt = ppool.tile([64, 2, 224], F32, name="pt", tag="pt")
                        for s in range(3):
                            nc.tensor.matmul(
                                pt[:, :, :],
                                wt_sb[:, s, :],
                                src[:, jd0 : jd0 + 2, s : s + 224],
                                start=(s == 0),
                                stop=(s == 2),
                            )
                        # G = 2*pi + ho == (partition base 64*pi + 32*ho)/32, so
                        # one 64-wide op per pair covers both ho groups
                        dst = ysb[64 * pi : 64 * (pi + 1), 2 * i : 2 * i + 2, :]
                        if i % 2 == 0:
                            nc.scalar.activation(
                                dst,
                                pt[:, :, :],
                                AF.Identity,
                                bias=b_sb[64 * pi : 64 * (pi + 1), :],
                            )
                        else:
                            nc.vector.tensor_scalar_add(
                                dst, pt[:, :, :], b_sb[64 * pi : 64 * (pi + 1), :]
                            )
                        # store finished m-halves while later pairs compute;
                        # (m, w) merge into one >=6KB descriptor per partition
                        # on the (otherwise idle) gpsimd SWDGE: descriptor
                        # generation on the ACT/sync rings would serialize
                        # with the evictions / the X loads.  The very last
                        # parity goes out in finer chunks on the (idle by
                        # then) sync ring to shorten the drain tail.
                        mtail = (
                            (MH, 20, 2 * MH)
                            if (pi and n == NB - 1 and t == NSLICE - 1)
                            else (MH, 2 * MH)
                        )
                        if 2 * i + 2 in mtail:
                            ci = mtail.index(2 * i + 2)
                            mlo = (0,) + mtail[:-1]
                            mlo, mhi = mlo[ci], mtail[ci]
                            eng = nc.sync if len(mtail) == 3 and ci else nc.gpsimd
                            eng.dma_start(
                                out=Y[n, 2 * pi : 2 * pi + 2, :,
                                      RS // 4 * t + mlo : RS // 4 * t + mhi, :],
                                in_=ysb[64 * pi : 64 * (pi + 1), mlo : mhi, :],
                            )


def build_nc(nb=NB, repeat=1):
    assert nb == NB
    nc = bacc.Bacc("TRN2", target_bir_lowering=False, debug=False)
    # X[n, q, c, jq, w] holds padded row 4*jq + q
    X = nc.dram_tensor("X", [NB, 4, C, NQ, WP], F16, kind="ExternalInput").ap()
    Wt = nc.dram_tensor("Wt", [128, 3, 64], F16, kind="ExternalInput").ap()
    Bias = nc.dram_tensor("bias", [128, 1], F32, kind="ExternalInput").ap()
    # staged fp16 output: Y[n, G, k, m, w] = conv(n, k, 4*m + G, w)
    Y = nc.dram_tensor("Y", [NB, 4, K, H // 4, W], F16, kind="ExternalOutput").ap()
    with tile.TileContext(nc) as tc:
        if repeat == 1:
            conv_body(tc, X, Wt, Bias, Y)
        else:
            with tc.For_i(0, repeat, 1):
                conv_body(tc, X, Wt, Bias, Y)
    nc.compile()
    return nc


def prep_weights(Wf, b):
    """Wt[32*q+c, s, 32*ho+k] = W[k, c, q-ho, s] (0 outside 0<=r<3)."""
    Wf = np.asarray(Wf, np.float32)
    Wt = np.zeros((128, 3, 64), np.float32)
    for q in range(4):
        for ho in range(2):
            r = q - ho
            if 0 <= r <= 2:
                Wt[32 * q : 32 * q + 32, :, 32 * ho : 32 * ho + 32] = Wf[
                    :, :, r, :
                ].transpose(1, 2, 0)
    bias = np.tile(np.asarray(b, np.float32), 4).reshape(128, 1)
    return Wt.astype(_NP16), bias


def pad_input(X):
    """Pad to 228x226 and pre-rotate rows: out[n, q, c, jd, w] = row 4*jd + q."""
    X = np.ascontiguousarray(X, np.float32)
    Xp = np.zeros((X.shape[0], C, H + 4, WP), _NP16)
    Xp[:, :, 1 : H + 1, 1 : W + 1] = X
    Xr = Xp.reshape(X.shape[0], C, NQ, 4, WP).transpose(0, 3, 1, 2, 4)
    return np.ascontiguousarray(Xr)


_NC = None


def _get_nc():
    global _NC
    if _NC is None:
        _NC = build_nc(NB)
    return _NC


def kernel(X, W, b, _trace=False):
    Xp = pad_input(X)
    Wt, bias = prep_weights(W, b)
    nc = _get_nc()
    in_maps = [
        {"X": Xp[NB * c : NB * (c + 1)], "Wt": Wt, "bias": bias} for c in range(NCORES)
    ]
    res = run_bass_kernel_spmd(nc, in_maps, list(range(NCORES)), trace=_trace)
    # un-interleave the staged layout: Y[n, k, 4*m + G, w] = staged[n, G, k, m, w]
    staged = np.concatenate([res.results[c]["Y"] for c in range(NCORES)], axis=0)
    out = np.ascontiguousarray(
        staged.transpose(0, 2, 3, 1, 4).reshape(NCORES * NB, 32, 224, 224),
        dtype=np.float32,
    )
    if _trace:
        return out, res
    return out
